# revision 15
# baseline (speedup 1.0000x reference)
"""Causal (cumulative) layer norm kernel for Trainium2, 8 NeuronCores.

Reference semantics (per (b, c) channel, running stats over time t):
    mean_t = cumsum(x)[t] / (t+1)
    var_t  = cumsum(x^2)[t] / (t+1) - mean_t^2
    out    = (x - mean_t) * rsqrt(var_t + 1e-5) * weight + bias

Sharding: data-parallel over batch B=8 -> one batch per core.

v3 design (per core, [T=4096, C=512], t-within-block on partitions):
  - f16 I/O: x downcast to f16 on host (plus an exact fp32 copy of the
    first 128 rows for the cancellation-sensitive small-t region); y is
    computed/stored f16 and upcast on host. Halves HBM traffic vs fp32.
  - two-level scan, waves of 8 blocks of 128 rows. Per wave, one-hot
    blocksum matmuls accumulate [sum x | sum x^2] rows plus a running
    total row (chained across waves by a K=1 matmul); one ACT copy moves
    them to SBUF f16. Carry selectors read the blocksum rows directly.
  - blocks 6..31 (n >= 768) use the xm-direct + m^2-interp form, which
    needs only TWO pointwise ops per block:
      left PSUM half:  xm = (I - U*inv_n) @ x - inv_n*carry  = x - mean
      right PSUM half: v  = U*inv_n @ x^2 + inv_n*qcarry - interp(m^2)
        (the linear interpolation of m^2 between block-boundary anchors
        is folded into the carry-selector matmul as 9 extra K rows; the
        within-block drift of m^2 is O(m*dm) < 0.005, measured max rel
        out err 0.0068 at block 6, decaying to 0.002 by block 16)
      rstd = Rsqrt(v + eps)    [ACT, PSUM->f16]
      out  = xm * rstd         [DVE]
    m^2 anchors per wave come from exclusive-carry matmuls (LSEL) and
    two tiny [9,512] ops (scaled ACT copy + DVE square).
  - blocks 1..5 use the exact path: scaled mean/q scans + carry
    selectors, then m2 = Square(mean) [ACT], xm = x - mean [DVE],
    v = q - m2 [DVE], rstd [ACT], out = xm*rstd [Pool].
  - block 0 (t < 128) runs the baseline's exact fp32 path: 3-way bf16
    splits of fp32 x and x^2 feed bit-faithful raw S/Q scans; pointwise
    uses n-scaled scalar_tensor_tensor ops.
  - every matmul output is <= 512 fp32 columns (PSUM bank limit).
"""
import numpy as np

EPS = 1e-5
B, T, C = 8, 4096, 512
P = 128                 # partitions / block size
NBLK = T // P           # 32
WAVE = 8                # blocks per wave
NWAVE = NBLK // WAVE    # 4
W2 = 2 * C              # combined [x | x^2] width
IST = 32                # first xm-direct/interp block (32 = disabled)

_CACHE = {}


def _build_consts():
    import ml_dtypes

    t_idx = np.arange(NBLK * P, dtype=np.float64).reshape(NBLK, P)
    inv_n = 1.0 / (t_idx + 1.0)            # [blk, p]
    U = np.triu(np.ones((P, P), np.float64), k=0)

    NI = NBLK - IST                        # interp blocks
    o_uinv = 0                             # 31 x [128,128] scaled scans
    o_hot = o_uinv + 31 * P                # 8 x [128,9]
    o_e8 = o_hot + 8 * 9                   # [1,9]
    o_sel = o_e8 + 9                       # 31 x [9,128]
    o_a = o_sel + 31 * P                   # NI x [128,128] xm-direct
    o_seln = o_a + NI * P                  # NI x [9,128] negated sel
    o_qsi = o_seln + NI * P                # NI x [18,128] qsel + m2 interp
    o_lsel = o_qsi + NI * P                # [9,9] exclusive carries
    RW = o_lsel + 9
    rb = np.zeros((P, RW), dtype=np.float16)

    for b in range(1, NBLK):
        w = U * inv_n[b][None, :]
        rb[:, o_uinv + (b - 1) * P:o_uinv + b * P] = w.astype(np.float16)
    # bs rows: row 0 = running total (chained), rows 1+i = block i sums
    for i in range(WAVE):
        rb[:, o_hot + 9 * i + 1 + i] = 1.0
        rb[:, o_hot + 9 * i + 0] = 1.0
    rb[0, o_e8 + 0] = 1.0
    # SEL(b): +carry*inv_n  (carry = row0 - sum(rows 1+i..8))
    for b in range(1, NBLK):
        i = b % WAVE
        s = o_sel + (b - 1) * P
        selw = inv_n[b].astype(np.float16)
        rb[0, s:s + P] = selw
        for k in range(i, WAVE):
            rb[1 + k, s:s + P] = -selw
    # A(b) = I - U*inv_n  (xm-direct), b in IST..31
    for b in range(IST, NBLK):
        a = np.eye(P) - U * inv_n[b][None, :]
        rb[:, o_a + (b - IST) * P:o_a + (b - IST + 1) * P] = \
            a.astype(np.float16)
    # SELN(b) = -SEL(b)
    for b in range(IST, NBLK):
        i = b % WAVE
        s = o_seln + (b - IST) * P
        selw = inv_n[b].astype(np.float16)
        rb[0, s:s + P] = -selw
        for k in range(i, WAVE):
            rb[1 + k, s:s + P] = selw
    # QSELI(b): rows 0-8 = SEL(b) (q carry), rows 32-40 = -interp(m^2)
    # (engine writes must start at partition 0/32/64, so the anchor rows
    # sit at base 32 and lhsT rows 9-31 stay zero)
    alpha = (1.0 - (np.arange(P) + 1.0) / P).astype(np.float16)
    beta = ((np.arange(P) + 1.0) / P).astype(np.float16)
    for b in range(IST, NBLK):
        i = b % WAVE
        s = o_qsi + (b - IST) * P
        selw = inv_n[b].astype(np.float16)
        rb[0, s:s + P] = selw
        for k in range(i, WAVE):
            rb[1 + k, s:s + P] = -selw
        rb[32 + i, s:s + P] = -alpha
        rb[33 + i, s:s + P] = -beta
    # LSEL [9,9]: col i = exclusive carry of wave-block i; col 8 = total
    for i in range(WAVE):
        rb[0, o_lsel + i] = 1.0
        for k in range(i, WAVE):
            rb[1 + k, o_lsel + i] = -1.0
    rb[0, o_lsel + 8] = 1.0

    # fblob f32 [128, 7]: block-0 scalars + per-wave 1/n0 anchor columns
    fb = np.zeros((P, 3 + NWAVE), dtype=np.float32)
    n0 = t_idx[0] + 1.0
    fb[:, 0] = (-inv_n[0]).astype(np.float32)       # -1/n
    fb[:, 1] = n0.astype(np.float32)                # n
    fb[:, 2] = (EPS * n0 * n0).astype(np.float32)   # eps*n^2
    for w in range(NWAVE):
        for i in range(9):
            nn = 128.0 * (w * WAVE + i)
            fb[i, 3 + w] = 0.0 if nn == 0 else 1.0 / nn

    # bblob bf16 [128, 128]: unscaled upper-tri scan matrix for block 0
    import ml_dtypes
    bb = U.astype(ml_dtypes.bfloat16)

    offs = {"uinv": o_uinv, "hot": o_hot, "e8": o_e8, "sel": o_sel,
            "a": o_a, "seln": o_seln, "qsi": o_qsi, "lsel": o_lsel, "w": RW}
    return fb, rb, bb, offs


def _build_program(iters=1):
    import concourse.bacc as bacc
    import concourse.tile as tile
    from concourse import mybir

    dt = mybir.dt
    AF = mybir.ActivationFunctionType
    ALU = mybir.AluOpType

    _, _, _, offs = _build_consts()
    RW = offs["w"]

    nc = bacc.Bacc()
    xh_d = nc.declare_dram_parameter("xh", [T, C], dt.float16, isOutput=False)
    x0_d = nc.declare_dram_parameter("x0", [P, C], dt.float32, isOutput=False)
    rb_d = nc.declare_dram_parameter("rblob", [P, RW], dt.float16, isOutput=False)
    fb_d = nc.declare_dram_parameter("fblob", [P, 3 + NWAVE], dt.float32,
                                     isOutput=False)
    bb_d = nc.declare_dram_parameter("bblob", [P, P], dt.bfloat16, isOutput=False)
    y_d = nc.declare_dram_parameter("y", [T, C], dt.float16, isOutput=True)

    x_v = xh_d[:, :].rearrange("(n p) c -> p n c", p=P)   # [128, 32, 512]
    y_v = y_d[:, :].rearrange("(n p) c -> p n c", p=P)

    def raw_rsqrt(out_ap, in_ap, bias_ap):
        eng = nc.scalar
        ins = [eng.lower_ap(in_ap), eng.lower_ap(bias_ap),
               mybir.ImmediateValue(dtype=dt.float32, value=1.0),
               mybir.ImmediateValue(dtype=dt.float32, value=0.0)]
        return eng.add_instruction(mybir.InstActivation(
            name=nc.get_next_instruction_name(), func=AF.Rsqrt,
            ins=ins, outs=[eng.lower_ap(out_ap)]))

    with tile.TileContext(nc) as tc:
        with (
            tc.tile_pool(name="consts", bufs=1) as consts,
            tc.tile_pool(name="waves", bufs=4) as waves,
            tc.tile_pool(name="bsp", bufs=4) as bsp,
            tc.tile_pool(name="blk", bufs=8) as blk,
            tc.tile_pool(name="blk1", bufs=1) as blk1,
            tc.tile_pool(name="ps_mq", bufs=3, space="PSUM") as ps_mq,
            tc.tile_pool(name="ps_small", bufs=1, space="PSUM") as ps_small,
        ):
            rb = consts.tile([P, RW], dt.float16, tag="rb")
            fb = consts.tile([P, 3 + NWAVE], dt.float32, tag="fb")
            bb = consts.tile([P, P], dt.bfloat16, tag="bb")
            nc.sync.dma_start(out=rb, in_=rb_d[:, :])
            nc.sync.dma_start(out=fb, in_=fb_d[:, :])
            nc.sync.dma_start(out=bb, in_=bb_d[:, :])
            eps_t = consts.tile([P, 1], dt.float32, tag="eps")
            nc.vector.memset(eps_t, EPS)

            neginv0 = fb[:, 0:1]
            nvec0 = fb[:, 1:2]
            epsn20 = fb[:, 2:3]

            def Uinv(b):        # [128,128] scaled scan lhsT, b in 1..31
                return rb[:, offs["uinv"] + (b - 1) * P:offs["uinv"] + b * P]

            def HOT(i):         # [128,9] one-hot col 1+i + ones col 0
                return rb[:, offs["hot"] + 9 * i:offs["hot"] + 9 * (i + 1)]

            E8 = rb[:, offs["e8"]:offs["e8"] + 9]

            def SEL(b):         # [128,128] carry selector (rows 9+ zero)
                return rb[:, offs["sel"] + (b - 1) * P:
                          offs["sel"] + b * P]

            def A(b):           # [128,128] xm-direct lhsT, b in IST..31
                return rb[:, offs["a"] + (b - IST) * P:
                          offs["a"] + (b - IST + 1) * P]

            def SELN(b):        # [9,128] negated carry selector
                return rb[0:9, offs["seln"] + (b - IST) * P:
                          offs["seln"] + (b - IST + 1) * P]

            def QSI(b):         # [41,128] q carry + m^2 interp
                return rb[0:41, offs["qsi"] + (b - IST) * P:
                          offs["qsi"] + (b - IST + 1) * P]

            LSEL = rb[0:9, offs["lsel"]:offs["lsel"] + 9]

            bs_tiles = []
            for k in range(NWAVE):
                t = consts.tile([P, W2], dt.float16, tag=f"bst{k}")
                nc.vector.memset(t, 0.0)
                bs_tiles.append(t)
            bs_ctr = [0]

            import contextlib

            # -- software-pipelined wave machinery ------------------------
            # Stage m1..m6 for wave wn are emitted interleaved into wave
            # w = wn-1's per-block work so the serial chain (bs matmuls ->
            # ACT copy -> carry matmuls -> ACT scale-copy -> m^2 square)
            # hides behind the previous wave's compute. The wave-3 step
            # prepares wave 0 of the NEXT For_i iteration (no chaining:
            # wave 0 starts fresh totals), and buffer-pool parity (4
            # allocations per body, bufs=2) makes the rotated tiles land
            # in the same slots the loop head reads.
            st = {}

            def m1_dma(wn):
                xq = waves.tile([P, WAVE, W2], dt.float16, tag="xq")
                nc.sync.dma_start(
                    out=xq[:, :, 0:C],
                    in_=x_v[:, wn * WAVE:(wn + 1) * WAVE, :])
                st[wn] = {"xq": xq}

            def m2_sq(wn):
                xq = st[wn]["xq"]
                nc.vector.tensor_tensor(
                    out=xq[:, :, C:W2], in0=xq[:, :, 0:C],
                    in1=xq[:, :, 0:C], op=ALU.mult)

            def m3_bs(wn, prev_bs):
                xq = st[wn]["xq"]
                bs_ps = ps_small.tile([9, W2], dt.float32, tag="small")
                for i in range(WAVE):
                    for h in range(2):
                        nc.tensor.matmul(bs_ps[:, h * C:(h + 1) * C], HOT(i),
                                         xq[:, i, h * C:(h + 1) * C],
                                         start=(i == 0),
                                         stop=(i == WAVE - 1 and
                                               prev_bs is None))
                if prev_bs is not None:
                    for h in range(2):
                        nc.tensor.matmul(bs_ps[:, h * C:(h + 1) * C], E8,
                                         prev_bs[:, h * C:(h + 1) * C],
                                         start=False, stop=True)
                st[wn]["bs_ps"] = bs_ps

            def m4_copy(wn):
                bs_ps = st[wn]["bs_ps"]
                bs_sb = bs_tiles[bs_ctr[0] % NWAVE]
                bs_ctr[0] += 1
                nc.scalar.copy(out=bs_sb[0:9, :], in_=bs_ps)
                st[wn]["bs_sb"] = bs_sb
                st[wn]["qrhs"] = None
                if IST < NBLK:
                    qrhs = bsp.tile([41, C], dt.float16, tag="qrhs")
                    nc.scalar.copy(out=qrhs[0:9, :], in_=bs_ps[:, C:W2])
                    st[wn]["qrhs"] = qrhs

            def m5_carr(wn):
                bs_sb = st[wn]["bs_sb"]
                ca_ps = ps_small.tile([9, W2], dt.float32, tag="small")
                for h in range(2):
                    nc.tensor.matmul(ca_ps[:, h * C:(h + 1) * C], LSEL,
                                     bs_sb[:, h * C:(h + 1) * C],
                                     start=True, stop=True)
                st[wn]["ca_ps"] = ca_ps

            def m6_anchor(wn):
                ca_ps = st[wn]["ca_ps"]
                qrhs = st[wn]["qrhs"]
                ma = bsp.tile([9, C], dt.float16, tag="ma")
                nc.scalar.activation(out=ma, in_=ca_ps[:, 0:C],
                                     func=AF.Copy,
                                     scale=fb[0:9, 3 + wn:4 + wn])
                nc.gpsimd.tensor_tensor(out=qrhs[32:41, :], in0=ma,
                                        in1=ma, op=ALU.mult)

            def machinery(wn, prev_bs):
                m1_dma(wn); m2_sq(wn); m3_bs(wn, prev_bs)
                m4_copy(wn)
                if IST < NBLK:
                    m5_carr(wn); m6_anchor(wn)

            def emit_block(w, i, blk0_parts):
                b = w * WAVE + i
                xq = st[w]["xq"]
                bs_sb = st[w]["bs_sb"]
                qrhs = st[w]["qrhs"]
                mq = ps_mq.tile([P, W2], dt.float32, tag="mq")
                if b == 0:
                    x0f, x0_3, sq0_3 = blk0_parts
                    for j, part in enumerate(x0_3):
                        nc.tensor.matmul(mq[:, 0:C], bb, part,
                                         start=(j == 0), stop=(j == 2))
                    for j, part in enumerate(sq0_3):
                        nc.tensor.matmul(mq[:, C:W2], bb, part,
                                         start=(j == 0), stop=(j == 2))
                    s_ps = mq[:, 0:C]
                    q_ps = mq[:, C:W2]
                    xm0 = blk1.tile([P, C], dt.float32, tag="xm0")
                    nc.vector.scalar_tensor_tensor(
                        out=xm0, in0=s_ps, scalar=neginv0, in1=x0f,
                        op0=ALU.mult, op1=ALU.add)
                    s2_0 = blk1.tile([P, C], dt.float32, tag="s2_0")
                    nc.scalar.square(out=s2_0, in_=s_ps)
                    d0 = blk1.tile([P, C], dt.float32, tag="d0")
                    nc.vector.scalar_tensor_tensor(
                        out=d0, in0=q_ps, scalar=nvec0, in1=s2_0,
                        op0=ALU.mult, op1=ALU.subtract)
                    r0 = blk1.tile([P, C], dt.float32, tag="r0")
                    raw_rsqrt(r0, d0, epsn20)
                    out0 = blk1.tile([P, C], dt.float16, tag="out0")
                    nc.vector.scalar_tensor_tensor(
                        out=out0, in0=xm0, scalar=nvec0, in1=r0,
                        op0=ALU.mult, op1=ALU.mult)
                    nc.sync.dma_start(out=y_v[:, 0, :], in_=out0)
                    return
                if b >= IST:
                    nc.tensor.matmul(mq[:, 0:C], A(b), xq[:, i, 0:C],
                                     start=True, stop=False)
                    nc.tensor.matmul(mq[:, 0:C], SELN(b), bs_sb[:, 0:C],
                                     start=False, stop=True)
                    nc.tensor.matmul(mq[:, C:W2], Uinv(b), xq[:, i, C:W2],
                                     start=True, stop=False)
                    nc.tensor.matmul(mq[:, C:W2], QSI(b), qrhs,
                                     start=False, stop=True)
                    rstd = blk.tile([P, C], dt.float16, tag="rstd")
                    raw_rsqrt(rstd, mq[:, C:W2], eps_t[:, :])
                    outp = blk.tile([P, C], dt.float16, tag="outp")
                    nc.vector.tensor_tensor(out=outp, in0=mq[:, 0:C],
                                            in1=rstd, op=ALU.mult)
                    nc.sync.dma_start(out=y_v[:, b, :], in_=outp)
                    return
                for h in range(2):
                    nc.tensor.matmul(mq[:, h * C:(h + 1) * C], Uinv(b),
                                     xq[:, i, h * C:(h + 1) * C],
                                     start=True, stop=False)
                for h in range(2):
                    nc.tensor.matmul(mq[:, h * C:(h + 1) * C], SEL(b),
                                     bs_sb[:, h * C:(h + 1) * C],
                                     start=False, stop=True)
                mean_ps = mq[:, 0:C]
                q_ps = mq[:, C:W2]
                m2 = blk.tile([P, C], dt.float16, tag="m2")
                nc.scalar.square(out=m2, in_=mean_ps)
                xm = blk.tile([P, C], dt.float16, tag="xm")
                nc.vector.tensor_tensor(out=xm, in0=xq[:, i, 0:C],
                                        in1=mean_ps, op=ALU.subtract)
                v = blk.tile([P, C], dt.float16, tag="v")
                nc.vector.tensor_tensor(out=v, in0=q_ps, in1=m2,
                                        op=ALU.subtract)
                rstd = blk.tile([P, C], dt.float16, tag="rstd")
                raw_rsqrt(rstd, v, eps_t[:, :])
                outp = blk.tile([P, C], dt.float16, tag="outp")
                nc.gpsimd.tensor_tensor(out=outp, in0=xm, in1=rstd,
                                        op=ALU.mult)
                nc.sync.dma_start(out=y_v[:, b, :], in_=outp)

            # prologue: wave-0 machinery for the first iteration
            machinery(0, None)

            loop_cm = tc.For_i(0, iters, 1) if iters > 1 else \
                contextlib.nullcontext()
            with loop_cm:
                for w in range(NWAVE):
                    wn = (w + 1) % NWAVE
                    blk0_parts = None
                    if w == 0:
                        # block 0 exact fp32 inputs
                        x0f = blk1.tile([P, C], dt.float32, tag="x0f")
                        nc.sync.dma_start(out=x0f, in_=x0_d[:, :])
                        sq0 = blk1.tile([P, C], dt.float32, tag="sq0")
                        nc.scalar.square(out=sq0, in_=x0f)

                        def split3(src, pfx):
                            hi = blk1.tile([P, C], dt.bfloat16, tag=pfx + "h")
                            nc.vector.tensor_copy(out=hi, in_=src)
                            r = blk1.tile([P, C], dt.float32, tag=pfx + "r")
                            nc.vector.tensor_tensor(out=r, in0=src, in1=hi,
                                                    op=ALU.subtract)
                            mid = blk1.tile([P, C], dt.bfloat16, tag=pfx + "m")
                            nc.vector.tensor_copy(out=mid, in_=r)
                            lo = blk1.tile([P, C], dt.bfloat16, tag=pfx + "l")
                            nc.vector.tensor_tensor(out=lo, in0=r, in1=mid,
                                                    op=ALU.subtract)
                            return hi, mid, lo

                        blk0_parts = (x0f, split3(x0f, "x0"),
                                      split3(sq0, "q0"))
                    m1_dma(wn)
                    emit_block(w, 0, blk0_parts)
                    m2_sq(wn)
                    emit_block(w, 1, None)
                    emit_block(w, 2, None)
                    m3_bs(wn, st[w]["bs_sb"] if wn != 0 else None)
                    emit_block(w, 3, None)
                    m4_copy(wn)
                    emit_block(w, 4, None)
                    if IST < NBLK:
                        m5_carr(wn)
                    emit_block(w, 5, None)
                    if IST < NBLK:
                        m6_anchor(wn)
                    emit_block(w, 6, None)
                    emit_block(w, 7, None)
    nc.compile()
    return nc


def kernel(x, weight, bias):
    from concourse.bass_utils import run_bass_kernel_spmd

    x = np.asarray(x, dtype=np.float32)
    w = np.asarray(weight, dtype=np.float32).reshape(-1)
    b = np.asarray(bias, dtype=np.float32).reshape(-1)

    if "nc" not in _CACHE:
        fb, rb, bb, _ = _build_consts()
        _CACHE["nc"] = _build_program()
        _CACHE["consts"] = {"rblob": rb, "fblob": fb, "bblob": bb}
    nc = _CACHE["nc"]

    xh = x.astype(np.float16)
    in_maps = [{"xh": np.ascontiguousarray(xh[core]),
                "x0": np.ascontiguousarray(x[core, 0:P]),
                **_CACHE["consts"]} for core in range(B)]
    res = run_bass_kernel_spmd(nc, in_maps, list(range(B)))
    y = np.stack([res.results[core]["y"] for core in range(B)], axis=0)
    y = y.astype(np.float32)

    trivial = np.all(w == 1.0) and np.all(b == 0.0)
    if not trivial:
        y = y * w[None, None, :] + b[None, None, :]
    return y


# revision 16
# speedup vs baseline: 1.0495x; 1.0495x over previous
"""Causal (cumulative) layer norm kernel for Trainium2, 8 NeuronCores.

Reference semantics (per (b, c) channel, running stats over time t):
    mean_t = cumsum(x)[t] / (t+1)
    var_t  = cumsum(x^2)[t] / (t+1) - mean_t^2
    out    = (x - mean_t) * rsqrt(var_t + 1e-5) * weight + bias

Sharding: data-parallel over batch B=8 -> one batch per core.

v3 design (per core, [T=4096, C=512], t-within-block on partitions):
  - f16 I/O: x downcast to f16 on host (plus an exact fp32 copy of the
    first 128 rows for the cancellation-sensitive small-t region); y is
    computed/stored f16 and upcast on host. Halves HBM traffic vs fp32.
  - two-level scan, waves of 8 blocks of 128 rows. Per wave, one-hot
    blocksum matmuls accumulate [sum x | sum x^2] rows plus a running
    total row (chained across waves by a K=1 matmul); one ACT copy moves
    them to SBUF f16. Carry selectors read the blocksum rows directly.
  - blocks 6..31 (n >= 768) use the xm-direct + m^2-interp form, which
    needs only TWO pointwise ops per block:
      left PSUM half:  xm = (I - U*inv_n) @ x - inv_n*carry  = x - mean
      right PSUM half: v  = U*inv_n @ x^2 + inv_n*qcarry - interp(m^2)
        (the linear interpolation of m^2 between block-boundary anchors
        is folded into the carry-selector matmul as 9 extra K rows; the
        within-block drift of m^2 is O(m*dm) < 0.005, measured max rel
        out err 0.0068 at block 6, decaying to 0.002 by block 16)
      rstd = Rsqrt(v + eps)    [ACT, PSUM->f16]
      out  = xm * rstd         [DVE]
    m^2 anchors per wave come from exclusive-carry matmuls (LSEL) and
    two tiny [9,512] ops (scaled ACT copy + DVE square).
  - blocks 1..5 use the exact path: scaled mean/q scans + carry
    selectors, then m2 = Square(mean) [ACT], xm = x - mean [DVE],
    v = q - m2 [DVE], rstd [ACT], out = xm*rstd [Pool].
  - block 0 (t < 128) runs the baseline's exact fp32 path: 3-way bf16
    splits of fp32 x and x^2 feed bit-faithful raw S/Q scans; pointwise
    uses n-scaled scalar_tensor_tensor ops.
  - every matmul output is <= 512 fp32 columns (PSUM bank limit).
"""
import numpy as np

EPS = 1e-5
B, T, C = 8, 4096, 512
P = 128                 # partitions / block size
NBLK = T // P           # 32
WAVE = 8                # blocks per wave
NWAVE = NBLK // WAVE    # 4
W2 = 2 * C              # combined [x | x^2] width
IST = 32                # first xm-direct/interp block (32 = disabled)

_CACHE = {}


def _build_consts():
    import ml_dtypes

    t_idx = np.arange(NBLK * P, dtype=np.float64).reshape(NBLK, P)
    inv_n = 1.0 / (t_idx + 1.0)            # [blk, p]
    U = np.triu(np.ones((P, P), np.float64), k=0)

    NI = NBLK - IST                        # interp blocks
    o_uinv = 0                             # 31 x [128,128] scaled scans
    o_hot = o_uinv + 31 * P                # 8 x [128,9]
    o_e8 = o_hot + 8 * 9                   # [1,9]
    o_sel = o_e8 + 9                       # 31 x [9,128]
    o_a = o_sel + 31 * P                   # NI x [128,128] xm-direct
    o_seln = o_a + NI * P                  # NI x [9,128] negated sel
    o_qsi = o_seln + NI * P                # NI x [18,128] qsel + m2 interp
    o_lsel = o_qsi + NI * P                # [9,9] exclusive carries
    RW = o_lsel + 9
    rb = np.zeros((P, RW), dtype=np.float16)

    for b in range(1, NBLK):
        w = U * inv_n[b][None, :]
        rb[:, o_uinv + (b - 1) * P:o_uinv + b * P] = w.astype(np.float16)
    # bs rows: row 0 = running total (chained), rows 1+i = block i sums
    for i in range(WAVE):
        rb[:, o_hot + 9 * i + 1 + i] = 1.0
        rb[:, o_hot + 9 * i + 0] = 1.0
    rb[0, o_e8 + 0] = 1.0
    # SEL(b): +carry*inv_n  (carry = row0 - sum(rows 1+i..8))
    for b in range(1, NBLK):
        i = b % WAVE
        s = o_sel + (b - 1) * P
        selw = inv_n[b].astype(np.float16)
        rb[0, s:s + P] = selw
        for k in range(i, WAVE):
            rb[1 + k, s:s + P] = -selw
    # A(b) = I - U*inv_n  (xm-direct), b in IST..31
    for b in range(IST, NBLK):
        a = np.eye(P) - U * inv_n[b][None, :]
        rb[:, o_a + (b - IST) * P:o_a + (b - IST + 1) * P] = \
            a.astype(np.float16)
    # SELN(b) = -SEL(b)
    for b in range(IST, NBLK):
        i = b % WAVE
        s = o_seln + (b - IST) * P
        selw = inv_n[b].astype(np.float16)
        rb[0, s:s + P] = -selw
        for k in range(i, WAVE):
            rb[1 + k, s:s + P] = selw
    # QSELI(b): rows 0-8 = SEL(b) (q carry), rows 32-40 = -interp(m^2)
    # (engine writes must start at partition 0/32/64, so the anchor rows
    # sit at base 32 and lhsT rows 9-31 stay zero)
    alpha = (1.0 - (np.arange(P) + 1.0) / P).astype(np.float16)
    beta = ((np.arange(P) + 1.0) / P).astype(np.float16)
    for b in range(IST, NBLK):
        i = b % WAVE
        s = o_qsi + (b - IST) * P
        selw = inv_n[b].astype(np.float16)
        rb[0, s:s + P] = selw
        for k in range(i, WAVE):
            rb[1 + k, s:s + P] = -selw
        rb[32 + i, s:s + P] = -alpha
        rb[33 + i, s:s + P] = -beta
    # LSEL [9,9]: col i = exclusive carry of wave-block i; col 8 = total
    for i in range(WAVE):
        rb[0, o_lsel + i] = 1.0
        for k in range(i, WAVE):
            rb[1 + k, o_lsel + i] = -1.0
    rb[0, o_lsel + 8] = 1.0

    # fblob f32 [128, 7]: block-0 scalars + per-wave 1/n0 anchor columns
    fb = np.zeros((P, 3 + NWAVE), dtype=np.float32)
    n0 = t_idx[0] + 1.0
    fb[:, 0] = (-inv_n[0]).astype(np.float32)       # -1/n
    fb[:, 1] = n0.astype(np.float32)                # n
    fb[:, 2] = (EPS * n0 * n0).astype(np.float32)   # eps*n^2
    for w in range(NWAVE):
        for i in range(9):
            nn = 128.0 * (w * WAVE + i)
            fb[i, 3 + w] = 0.0 if nn == 0 else 1.0 / nn

    # bblob bf16 [128, 128]: unscaled upper-tri scan matrix for block 0
    import ml_dtypes
    bb = U.astype(ml_dtypes.bfloat16)

    offs = {"uinv": o_uinv, "hot": o_hot, "e8": o_e8, "sel": o_sel,
            "a": o_a, "seln": o_seln, "qsi": o_qsi, "lsel": o_lsel, "w": RW}
    return fb, rb, bb, offs


def _build_program(iters=1):
    import concourse.bacc as bacc
    import concourse.tile as tile
    from concourse import mybir

    dt = mybir.dt
    AF = mybir.ActivationFunctionType
    ALU = mybir.AluOpType

    _, _, _, offs = _build_consts()
    RW = offs["w"]

    nc = bacc.Bacc()
    xh_d = nc.declare_dram_parameter("xh", [T, C], dt.float16, isOutput=False)
    x0_d = nc.declare_dram_parameter("x0", [P, C], dt.float32, isOutput=False)
    rb_d = nc.declare_dram_parameter("rblob", [P, RW], dt.float16, isOutput=False)
    fb_d = nc.declare_dram_parameter("fblob", [P, 3 + NWAVE], dt.float32,
                                     isOutput=False)
    bb_d = nc.declare_dram_parameter("bblob", [P, P], dt.bfloat16, isOutput=False)
    y_d = nc.declare_dram_parameter("y", [T, C], dt.float16, isOutput=True)

    x_v = xh_d[:, :].rearrange("(n p) c -> p n c", p=P)   # [128, 32, 512]
    y_v = y_d[:, :].rearrange("(n p) c -> p n c", p=P)

    def raw_rsqrt(out_ap, in_ap, bias_ap):
        eng = nc.scalar
        ins = [eng.lower_ap(in_ap), eng.lower_ap(bias_ap),
               mybir.ImmediateValue(dtype=dt.float32, value=1.0),
               mybir.ImmediateValue(dtype=dt.float32, value=0.0)]
        return eng.add_instruction(mybir.InstActivation(
            name=nc.get_next_instruction_name(), func=AF.Rsqrt,
            ins=ins, outs=[eng.lower_ap(out_ap)]))

    with tile.TileContext(nc) as tc:
        with (
            tc.tile_pool(name="consts", bufs=1) as consts,
            tc.tile_pool(name="waves", bufs=4) as waves,
            tc.tile_pool(name="bsp", bufs=4) as bsp,
            tc.tile_pool(name="blk", bufs=8) as blk,
            tc.tile_pool(name="blk1", bufs=1) as blk1,
            tc.tile_pool(name="ps_mq", bufs=3, space="PSUM") as ps_mq,
            tc.tile_pool(name="ps_small", bufs=1, space="PSUM") as ps_small,
        ):
            rb = consts.tile([P, RW], dt.float16, tag="rb")
            fb = consts.tile([P, 3 + NWAVE], dt.float32, tag="fb")
            bb = consts.tile([P, P], dt.bfloat16, tag="bb")
            nc.sync.dma_start(out=rb, in_=rb_d[:, :])
            nc.sync.dma_start(out=fb, in_=fb_d[:, :])
            nc.sync.dma_start(out=bb, in_=bb_d[:, :])
            eps_t = consts.tile([P, 1], dt.float32, tag="eps")
            nc.vector.memset(eps_t, EPS)

            neginv0 = fb[:, 0:1]
            nvec0 = fb[:, 1:2]
            epsn20 = fb[:, 2:3]

            def Uinv(b):        # [128,128] scaled scan lhsT, b in 1..31
                return rb[:, offs["uinv"] + (b - 1) * P:offs["uinv"] + b * P]

            def HOT(i):         # [128,9] one-hot col 1+i + ones col 0
                return rb[:, offs["hot"] + 9 * i:offs["hot"] + 9 * (i + 1)]

            E8 = rb[0:1, offs["e8"]:offs["e8"] + 9]

            def SEL(b):         # [9,128] carry selector, b in 1..31
                return rb[0:9, offs["sel"] + (b - 1) * P:
                          offs["sel"] + b * P]

            def A(b):           # [128,128] xm-direct lhsT, b in IST..31
                return rb[:, offs["a"] + (b - IST) * P:
                          offs["a"] + (b - IST + 1) * P]

            def SELN(b):        # [9,128] negated carry selector
                return rb[0:9, offs["seln"] + (b - IST) * P:
                          offs["seln"] + (b - IST + 1) * P]

            def QSI(b):         # [41,128] q carry + m^2 interp
                return rb[0:41, offs["qsi"] + (b - IST) * P:
                          offs["qsi"] + (b - IST + 1) * P]

            LSEL = rb[0:9, offs["lsel"]:offs["lsel"] + 9]


            import contextlib

            # -- software-pipelined wave machinery ------------------------
            # Stage m1..m6 for wave wn are emitted interleaved into wave
            # w = wn-1's per-block work so the serial chain (bs matmuls ->
            # ACT copy -> carry matmuls -> ACT scale-copy -> m^2 square)
            # hides behind the previous wave's compute. The wave-3 step
            # prepares wave 0 of the NEXT For_i iteration (no chaining:
            # wave 0 starts fresh totals), and buffer-pool parity (4
            # allocations per body, bufs=2) makes the rotated tiles land
            # in the same slots the loop head reads.
            st = {}

            def m1_dma(wn):
                xq = waves.tile([P, WAVE, W2], dt.float16, tag="xq")
                nc.sync.dma_start(
                    out=xq[:, :, 0:C],
                    in_=x_v[:, wn * WAVE:(wn + 1) * WAVE, :])
                st[wn] = {"xq": xq}

            def m2_sq(wn):
                xq = st[wn]["xq"]
                nc.vector.tensor_tensor(
                    out=xq[:, :, C:W2], in0=xq[:, :, 0:C],
                    in1=xq[:, :, 0:C], op=ALU.mult)

            def m3_bs(wn, prev_bs):
                xq = st[wn]["xq"]
                bs_ps = ps_small.tile([9, W2], dt.float32, tag="small")
                for i in range(WAVE):
                    for h in range(2):
                        nc.tensor.matmul(bs_ps[:, h * C:(h + 1) * C], HOT(i),
                                         xq[:, i, h * C:(h + 1) * C],
                                         start=(i == 0),
                                         stop=(i == WAVE - 1 and
                                               prev_bs is None))
                if prev_bs is not None:
                    for h in range(2):
                        nc.tensor.matmul(bs_ps[:, h * C:(h + 1) * C], E8,
                                         prev_bs[0:1, h * C:(h + 1) * C],
                                         start=False, stop=True)
                st[wn]["bs_ps"] = bs_ps

            def m4_copy(wn):
                bs_ps = st[wn]["bs_ps"]
                bs_sb = bsp.tile([9, W2], dt.float16, tag="bs")
                nc.scalar.copy(out=bs_sb, in_=bs_ps)
                st[wn]["bs_sb"] = bs_sb
                st[wn]["qrhs"] = None
                if IST < NBLK:
                    qrhs = bsp.tile([41, C], dt.float16, tag="qrhs")
                    nc.scalar.copy(out=qrhs[0:9, :], in_=bs_ps[:, C:W2])
                    st[wn]["qrhs"] = qrhs

            def m5_carr(wn):
                bs_sb = st[wn]["bs_sb"]
                ca_ps = ps_small.tile([9, W2], dt.float32, tag="small")
                for h in range(2):
                    nc.tensor.matmul(ca_ps[:, h * C:(h + 1) * C], LSEL,
                                     bs_sb[:, h * C:(h + 1) * C],
                                     start=True, stop=True)
                st[wn]["ca_ps"] = ca_ps

            def m6_anchor(wn):
                ca_ps = st[wn]["ca_ps"]
                qrhs = st[wn]["qrhs"]
                ma = bsp.tile([9, C], dt.float16, tag="ma")
                nc.scalar.activation(out=ma, in_=ca_ps[:, 0:C],
                                     func=AF.Copy,
                                     scale=fb[0:9, 3 + wn:4 + wn])
                nc.gpsimd.tensor_tensor(out=qrhs[32:41, :], in0=ma,
                                        in1=ma, op=ALU.mult)

            def machinery(wn, prev_bs):
                m1_dma(wn); m2_sq(wn); m3_bs(wn, prev_bs)
                m4_copy(wn)
                if IST < NBLK:
                    m5_carr(wn); m6_anchor(wn)

            def emit_block(w, i, blk0_parts):
                b = w * WAVE + i
                xq = st[w]["xq"]
                bs_sb = st[w]["bs_sb"]
                qrhs = st[w]["qrhs"]
                mq = ps_mq.tile([P, W2], dt.float32, tag="mq")
                if b == 0:
                    x0f, x0_3, sq0_3 = blk0_parts
                    for j, part in enumerate(x0_3):
                        nc.tensor.matmul(mq[:, 0:C], bb, part,
                                         start=(j == 0), stop=(j == 2))
                    for j, part in enumerate(sq0_3):
                        nc.tensor.matmul(mq[:, C:W2], bb, part,
                                         start=(j == 0), stop=(j == 2))
                    s_ps = mq[:, 0:C]
                    q_ps = mq[:, C:W2]
                    xm0 = blk1.tile([P, C], dt.float32, tag="xm0")
                    nc.vector.scalar_tensor_tensor(
                        out=xm0, in0=s_ps, scalar=neginv0, in1=x0f,
                        op0=ALU.mult, op1=ALU.add)
                    s2_0 = blk1.tile([P, C], dt.float32, tag="s2_0")
                    nc.scalar.square(out=s2_0, in_=s_ps)
                    d0 = blk1.tile([P, C], dt.float32, tag="d0")
                    nc.vector.scalar_tensor_tensor(
                        out=d0, in0=q_ps, scalar=nvec0, in1=s2_0,
                        op0=ALU.mult, op1=ALU.subtract)
                    r0 = blk1.tile([P, C], dt.float32, tag="r0")
                    raw_rsqrt(r0, d0, epsn20)
                    out0 = blk1.tile([P, C], dt.float16, tag="out0")
                    nc.vector.scalar_tensor_tensor(
                        out=out0, in0=xm0, scalar=nvec0, in1=r0,
                        op0=ALU.mult, op1=ALU.mult)
                    nc.sync.dma_start(out=y_v[:, 0, :], in_=out0)
                    return
                if b >= IST:
                    nc.tensor.matmul(mq[:, 0:C], A(b), xq[:, i, 0:C],
                                     start=True, stop=False)
                    nc.tensor.matmul(mq[:, 0:C], SELN(b), bs_sb[:, 0:C],
                                     start=False, stop=True)
                    nc.tensor.matmul(mq[:, C:W2], Uinv(b), xq[:, i, C:W2],
                                     start=True, stop=False)
                    nc.tensor.matmul(mq[:, C:W2], QSI(b), qrhs,
                                     start=False, stop=True)
                    rstd = blk.tile([P, C], dt.float16, tag="rstd")
                    raw_rsqrt(rstd, mq[:, C:W2], eps_t[:, :])
                    outp = blk.tile([P, C], dt.float16, tag="outp")
                    nc.vector.tensor_tensor(out=outp, in0=mq[:, 0:C],
                                            in1=rstd, op=ALU.mult)
                    nc.sync.dma_start(out=y_v[:, b, :], in_=outp)
                    return
                for h in range(2):
                    nc.tensor.matmul(mq[:, h * C:(h + 1) * C], Uinv(b),
                                     xq[:, i, h * C:(h + 1) * C],
                                     start=True, stop=False)
                for h in range(2):
                    nc.tensor.matmul(mq[:, h * C:(h + 1) * C], SEL(b),
                                     bs_sb[:, h * C:(h + 1) * C],
                                     start=False, stop=True)
                mean_ps = mq[:, 0:C]
                q_ps = mq[:, C:W2]
                m2 = blk.tile([P, C], dt.float16, tag="m2")
                nc.scalar.square(out=m2, in_=mean_ps)
                xm = blk.tile([P, C], dt.float16, tag="xm")
                nc.vector.tensor_tensor(out=xm, in0=xq[:, i, 0:C],
                                        in1=mean_ps, op=ALU.subtract)
                v = blk.tile([P, C], dt.float16, tag="v")
                nc.vector.tensor_tensor(out=v, in0=q_ps, in1=m2,
                                        op=ALU.subtract)
                rstd = blk.tile([P, C], dt.float16, tag="rstd")
                raw_rsqrt(rstd, v, eps_t[:, :])
                outp = blk.tile([P, C], dt.float16, tag="outp")
                nc.gpsimd.tensor_tensor(out=outp, in0=xm, in1=rstd,
                                        op=ALU.mult)
                nc.sync.dma_start(out=y_v[:, b, :], in_=outp)

            # prologue: wave-0 machinery for the first iteration
            machinery(0, None)

            loop_cm = tc.For_i(0, iters, 1) if iters > 1 else \
                contextlib.nullcontext()
            with loop_cm:
                for w in range(NWAVE):
                    wn = (w + 1) % NWAVE
                    blk0_parts = None
                    if w == 0:
                        # block 0 exact fp32 inputs
                        x0f = blk1.tile([P, C], dt.float32, tag="x0f")
                        nc.sync.dma_start(out=x0f, in_=x0_d[:, :])
                        sq0 = blk1.tile([P, C], dt.float32, tag="sq0")
                        nc.scalar.square(out=sq0, in_=x0f)

                        def split3(src, pfx):
                            hi = blk1.tile([P, C], dt.bfloat16, tag=pfx + "h")
                            nc.vector.tensor_copy(out=hi, in_=src)
                            r = blk1.tile([P, C], dt.float32, tag=pfx + "r")
                            nc.vector.tensor_tensor(out=r, in0=src, in1=hi,
                                                    op=ALU.subtract)
                            mid = blk1.tile([P, C], dt.bfloat16, tag=pfx + "m")
                            nc.vector.tensor_copy(out=mid, in_=r)
                            lo = blk1.tile([P, C], dt.bfloat16, tag=pfx + "l")
                            nc.vector.tensor_tensor(out=lo, in0=r, in1=mid,
                                                    op=ALU.subtract)
                            return hi, mid, lo

                        blk0_parts = (x0f, split3(x0f, "x0"),
                                      split3(sq0, "q0"))
                    m1_dma(wn)
                    emit_block(w, 0, blk0_parts)
                    m2_sq(wn)
                    emit_block(w, 1, None)
                    emit_block(w, 2, None)
                    m3_bs(wn, st[w]["bs_sb"] if wn != 0 else None)
                    emit_block(w, 3, None)
                    m4_copy(wn)
                    emit_block(w, 4, None)
                    if IST < NBLK:
                        m5_carr(wn)
                    emit_block(w, 5, None)
                    if IST < NBLK:
                        m6_anchor(wn)
                    emit_block(w, 6, None)
                    emit_block(w, 7, None)
    nc.compile()
    return nc


def kernel(x, weight, bias):
    from concourse.bass_utils import run_bass_kernel_spmd

    x = np.asarray(x, dtype=np.float32)
    w = np.asarray(weight, dtype=np.float32).reshape(-1)
    b = np.asarray(bias, dtype=np.float32).reshape(-1)

    if "nc" not in _CACHE:
        fb, rb, bb, _ = _build_consts()
        _CACHE["nc"] = _build_program()
        _CACHE["consts"] = {"rblob": rb, "fblob": fb, "bblob": bb}
    nc = _CACHE["nc"]

    xh = x.astype(np.float16)
    in_maps = [{"xh": np.ascontiguousarray(xh[core]),
                "x0": np.ascontiguousarray(x[core, 0:P]),
                **_CACHE["consts"]} for core in range(B)]
    res = run_bass_kernel_spmd(nc, in_maps, list(range(B)))
    y = np.stack([res.results[core]["y"] for core in range(B)], axis=0)
    y = y.astype(np.float32)

    trivial = np.all(w == 1.0) and np.all(b == 0.0)
    if not trivial:
        y = y * w[None, None, :] + b[None, None, :]
    return y
